# revision 91
# baseline (speedup 1.0000x reference)
"""Trainium2 Bass kernel for a post-LN transformer decoder layer.

Sharding: 8 cores = 4 batches x 2 token-halves. Core c=(b,j) handles batch b
and 2 query quarter-blocks (j=0: global rows [0:512)+[1536:2048),
j=1: [512:1536)) -- interleaved quarters balance the causal triangle.

Key optimizations over the naive pipeline:
- All projections (Q/K/V both attns) and the FFN run as fp8e4 DoubleRow
  matmuls (0.5 cycles/row): host quantizes x/enc/weights to e4m3 with
  power-of-2 scales; descales fold into the existing bias activations.
  Attention scores / attn-V matmuls stay bf16 (an all-fp8 attention was
  tried and made the whole PE clock-throttle harder -- net loss).
- V for all heads is computed upfront in [tokens, d] layout with 512-wide
  moving dims (2.25x fewer PE rows than per-head-pair V); V biases land in
  PSUM via a ones-row matmul so the SBUF copy is a pure scaled ACT copy.
- Exp is fused across each head pair (one ACT op per k-tile over a 2-bank
  PSUM tile); softmax reciprocals run as ONE lane-parallel DVE reciprocal
  per head-pair (rows 0/64), and the normalize epilogue is deferred into
  the next slot's score stream so the PE never stalls on it.
- Per-head-pair inputs (weight DMAs, K/Q projections, V copies) are
  emitted one head-pair ahead; enc DMAs + the Q2 projection are hoisted
  before LN1 so their PE work fills the LN1/V2/AllGather serial window;
  FFN weights prefetch one tile ahead and are loaded once.
- LN mean/var run via ones-column matmuls on bf16 copies (GpSimd casts),
  rstd uses a raw ACT Rsqrt (accuracy ample for LN), out-proj bias +
  residual add fuse into one DVE scalar_tensor_tensor.
- The V2 = x1 @ wv2 AllGather (the only collective) is split in two
  halves so cross-attention can consume the first half early.

Residual/LN path is fp32.  The program must be identical on all 8 cores
(single SPMD NEFF), so the causal structure is padded to a uniform (8, 16)
k-tile schedule per query slot and causal masking multiplies exp outputs
by per-core 0/1 ramp windows (two shared [128,1920] ramps, static offsets).
"""

import sys

sys.path.insert(0, "/opt/trn_rl_repo")

import numpy as np
import ml_dtypes

import os

import concourse.bass as bass
import concourse.tile as tile
from concourse import mybir
from concourse.bass_utils import run_bass_kernel_spmd

USE_FAST_RECIP = os.environ.get("K_FAST_RECIP", "0") == "1"
USE_GPSIMD = os.environ.get("K_GPSIMD", "1") == "1"

BF16 = mybir.dt.bfloat16
F8 = mybir.dt.float8e4
F32 = mybir.dt.float32
AF = mybir.ActivationFunctionType
DR = mybir.MatmulPerfMode.DoubleRow
MUL = mybir.AluOpType.mult
ADD = mybir.AluOpType.add

D = 1024       # d_model
DFF = 4096
B, S = 4, 2048
NCORES = 8
QL = 1024      # local query rows per core
EPS = 1e-6
MT = 8         # d_model 128-tiles
FT = 32        # d_ff 128-tiles
SCALE = 0.125  # 1/sqrt(head_dim)
SLOT_NKT = (8, 16)  # uniform k-tile count per query slot; last 8 are masked

SX = 32.0      # host fp8 scale for x / enc
SW = 4096.0    # host fp8 scale for d_model-input weights
SW2 = 8192.0   # host fp8 scale for wf2 (d_ff input)
SA = 16.0      # on-chip fp8 scale for LN outputs (x1, x2) and ffn hidden
SV = 16.0      # on-chip fp8 scale for V tiles (attn values)
SAT = 32.0     # on-chip fp8 scale for normalized attention outputs
SWO = 4096.0   # host fp8 scale for out-proj weights
DS_OW = 1.0 / (SAT * SWO)  # descale for attn @ wo
DS_XW = 1.0 / (SX * SW)    # descale for x/enc @ w projections
DS_AW = 1.0 / (SA * SW)    # descale for x1/x2 @ w projections
DS_H2 = 1.0 / (SA * SW2)   # descale for h @ wf2
MASK_NEG = -400.0          # additive mask on scores; exp(-50) == 0 in f32

# AllGather is per core-pair; pair-local rank j owns quarters (Q0,Q3) for j=0
# and (Q1,Q2) for j=1.  v2all row base for global k-tile t is
# V2_ROW0[t//4] + (t%4)*128.
V2_ROW0 = [0, 1024, 1536, 512]


def _eng(nc):
    return nc.gpsimd if USE_GPSIMD else nc.vector


def _act_raw(nc, out, in_, func, bias=0.0, scale=1.0):
    """Raw InstActivation emission: out = func(in_*scale + bias).

    Bypasses bass's Reciprocal/Rsqrt accuracy guard -- the attention
    normalizer and LN rstd only need ~1% relative accuracy, and the ACT
    LUT versions are ~5x faster than DVE's microcoded reciprocal."""
    eng = nc.scalar
    inputs = [eng.lower_ap(in_)]
    for arg in (bias, scale, 0.0):
        if isinstance(arg, bass.AP):
            inputs.append(eng.lower_ap(arg))
        else:
            inputs.append(mybir.ImmediateValue(dtype=mybir.dt.float32, value=arg))
    return eng.add_instruction(
        mybir.InstActivation(
            name=nc.get_next_instruction_name(),
            func=func,
            ins=inputs,
            outs=[eng.lower_ap(out)],
        )
    )


def legalize_waits(nc, max_waits=1):
    """This walrus build accepts at most one sync-wait per instruction.
    Hoist excess waits onto same-engine NoOps inserted just before."""
    nid = 0
    for fn in nc.m.functions:
        for bb in fn.blocks:
            new = []
            changed = False
            for inst in bb.instructions:
                si = inst.sync_info
                if si is not None and si.on_wait and len(si.on_wait) > max_waits:
                    waits = list(si.on_wait)
                    for w in waits[:-max_waits]:
                        nid += 1
                        nop = mybir.InstNoOp(name=f"I-waitfix-{nid}", ins=[], outs=[])
                        nop.engine = inst.engine
                        nop.sync_info = mybir.SyncInfo(on_wait=[w], on_update=[])
                        new.append(nop)
                    inst.sync_info = mybir.SyncInfo(
                        on_wait=waits[-max_waits:], on_update=list(si.on_update)
                    )
                    changed = True
                new.append(inst)
            if changed:
                bb.instructions = new


def build_nc():
    nc = bass.Bass(num_devices=NCORES)

    xT = nc.dram_tensor("xT", [D, S], F8, kind="ExternalInput")
    xTq = nc.dram_tensor("xTq", [D, QL], F8, kind="ExternalInput")
    xres = nc.dram_tensor("xres", [D, QL], F32, kind="ExternalInput")
    encT = nc.dram_tensor("encT", [D, S], F8, kind="ExternalInput")
    encTq = nc.dram_tensor("encTq", [D, QL], F8, kind="ExternalInput")
    wd = {}
    for name in ("wq1", "wk1", "wv1", "wq2", "wk2", "wv2"):
        wd[name] = nc.dram_tensor(name, [D, D], F8, kind="ExternalInput")
    for name in ("wo1", "wo2"):
        wd[name] = nc.dram_tensor(name, [D, D], F8, kind="ExternalInput")
    # out-proj bias rows, host-prescaled by SAT*SWO (added in PSUM via a
    # stationary-row x ones matmul, like the V biases)
    ro1 = nc.dram_tensor("ro1", [1, D], BF16, kind="ExternalInput")
    ro2 = nc.dram_tensor("ro2", [1, D], BF16, kind="ExternalInput")
    wd["wf1"] = nc.dram_tensor("wf1", [D, DFF], F8, kind="ExternalInput")
    wd["wf2"] = nc.dram_tensor("wf2", [DFF, D], F8, kind="ExternalInput")
    bias_specs = (
        ("cq1", 8), ("ck1", 8), ("co1", 8), ("cq2", 8), ("ck2", 8), ("co2", 8),
        ("cf1", 32), ("cf2", 8),
        ("g1", 8), ("be1", 8), ("g2", 8), ("be2", 8), ("g3", 8), ("be3", 8),
    )
    bcd = {}
    for name, k in bias_specs:
        bcd[name] = nc.dram_tensor(name, [128, k], F32, kind="ExternalInput")
    # V bias rows, host-prescaled to the V psum scale (SX*SW / SA*SW)
    rv1 = nc.dram_tensor("rv1", [1, D], BF16, kind="ExternalInput")
    rv2 = nc.dram_tensor("rv2", [1, D], BF16, kind="ExternalInput")
    # Causal masks for (slot, k-tile) are shifted windows of two per-slot
    # ramps: ramp[sl][i, u] = (u >= i + c(core, sl)); window offset for
    # (sl, ki) is (896, 1408)[sl] - 128*ki.
    bigmask = nc.dram_tensor("bigmask", [2, 128, 1920], BF16, kind="ExternalInput")
    outT = nc.dram_tensor("outT", [D, QL], F32, kind="ExternalOutput")
    # V2 AllGather is split in two halves so cross-attention can start on
    # the first half while the second is still in flight.  Half A carries
    # each core's qb0 token tiles (global tiles 0-7 in order), half B the
    # qb1 tiles (global 12-15 then 8-11).
    v2locA = nc.dram_tensor("v2locA", [QL // 2, D], BF16)
    v2locB = nc.dram_tensor("v2locB", [QL // 2, D], BF16)
    v2allA = nc.dram_tensor("v2allA", [QL, D], BF16)
    v2allB = nc.dram_tensor("v2allB", [QL, D], BF16)

    def wslices(name):
        return wd[name].ap().rearrange("(mt p) d -> p mt d", p=128)

    with tile.TileContext(nc) as tc:
        _cms = {}

        def open_pool(**kw):
            cm = tc.tile_pool(**kw)
            _cms[kw["name"]] = cm
            return cm.__enter__()

        def close_pool(pool_name):
            _cms.pop(pool_name).__exit__(None, None, None)

        # long-lived pools (whole kernel)
        const = open_pool(name="const", bufs=1)
        wpool = open_pool(name="wpool", bufs=2)
        hpool = open_pool(name="hpool", bufs=2)
        epool = open_pool(name="epool", bufs=4)
        sp1 = open_pool(name="sp1", bufs=1)    # LN row scratch
        sp2 = open_pool(name="sp2", bufs=2)    # attention normalize scratch
        lnsc = open_pool(name="lnsc", bufs=2)  # per-mt LN scratch
        # PSUM: s2 tag = [128,2,512] (2 banks) x2 bufs = 4 banks;
        # u tag = 1 bank x2; pp tag = 1 bank x2  -> exactly 8 banks.
        psS = open_pool(name="psS", bufs=2, space="PSUM")
        psU = open_pool(name="psU", bufs=2, space="PSUM")
        psP = open_pool(name="psP", bufs=2, space="PSUM")

        # ---- phase-A inputs first: the first PE work (V1) needs these,
        # and two dozen tiny bias DMAs would otherwise delay it ----
        pA = open_pool(name="pA", bufs=1)                    # xT/xTq/mask
        xTs = pA.tile([128, MT, S], F8, tag="xTs")
        for mt in range(MT):
            nc.sync.dma_start(
                out=xTs[:, mt, :],
                in_=xT.ap().rearrange("(mt p) s -> p mt s", p=128)[:, mt, :],
            )
        wv1s = wpool.tile([128, MT, D], F8, tag="wvfull", bufs=1)
        nc.sync.dma_start(out=wv1s, in_=wslices("wv1"))
        xqs = pA.tile([128, MT, QL], F8, tag="xqs")
        for mt in range(MT):
            nc.sync.dma_start(
                out=xqs[:, mt, :],
                in_=xTq.ap().rearrange("(mt p) s -> p mt s", p=128)[:, mt, :],
            )
        maskt = pA.tile([128, 2, 1920], BF16, tag="maskt")
        nc.sync.dma_start(
            out=maskt, in_=bigmask.ap().rearrange("sl p u -> p sl u")
        )

        # ---- constants ----
        bc = {}
        for name, k in bias_specs:
            t = const.tile([128, k], F32, tag=name)
            nc.sync.dma_start(out=t, in_=bcd[name][:, :])
            bc[name] = t
        rv1_sb = const.tile([1, D], BF16, tag="rv1")
        nc.sync.dma_start(out=rv1_sb, in_=rv1[:, :])
        rv2_sb = const.tile([1, D], BF16, tag="rv2")
        nc.sync.dma_start(out=rv2_sb, in_=rv2[:, :])
        ro1_sb = const.tile([1, D], BF16, tag="ro1")
        nc.sync.dma_start(out=ro1_sb, in_=ro1[:, :])
        ro2_sb = const.tile([1, D], BF16, tag="ro2")
        nc.sync.dma_start(out=ro2_sb, in_=ro2[:, :])
        ones_row = const.tile([1, 512], BF16, tag="ones_row")
        nc.vector.memset(ones_row, 1.0)
        # ones row living at partition 64: stationary for the h=1 normalize
        # broadcast (matmul requires stationary/moving partition bases match)
        ones64 = const.tile([65, 64], BF16, tag="ones64")
        nc.vector.memset(ones64[64:65, :], 1.0)
        ones_col = const.tile([128, 1], BF16, tag="ones_col")
        nc.vector.memset(ones_col, 1.0)
        epsr = const.tile([1, 1], F32, tag="epsr")
        nc.vector.memset(epsr, EPS)

        def ln_block(z, gname, bename, xout, xf8out, qb):
            """LayerNorm over the partition(d_model) axis of z [128, MT, 512]
            f32.  Writes xout[:, :, qb*512:+512] f32 (+ optional f8 copy
            scaled by SA)."""
            psum_s = psU.tile([65, 512], F32, tag="u")
            psum_q = psU.tile([65, 512], F32, tag="u")
            for mt in range(MT):
                zbf = lnsc.tile([128, 512], BF16, tag="zbf")
                _eng(nc).tensor_copy(zbf[:, :], z[:, mt, :])
                zsq = lnsc.tile([128, 512], BF16, tag="zsq")
                _eng(nc).tensor_mul(zsq[:, :], zbf[:, :], zbf[:, :])
                nc.tensor.matmul(psum_s[0:1, :], ones_col[:, :], zbf[:, :],
                                 start=(mt == 0), stop=(mt == MT - 1))
                nc.tensor.matmul(psum_q[0:1, :], ones_col[:, :], zsq[:, :],
                                 start=(mt == 0), stop=(mt == MT - 1))
            mu = sp1.tile([1, 512], F32, tag="ln_mu")
            nc.scalar.activation(mu[:, :], psum_s[0:1, :], AF.Copy, scale=1.0 / D)
            t = sp1.tile([1, 512], F32, tag="ln_t")
            nc.scalar.activation(t[:, :], psum_q[0:1, :], AF.Copy, scale=1.0 / D)
            musq = sp1.tile([1, 512], F32, tag="ln_musq")
            nc.vector.tensor_mul(musq[:, :], mu[:, :], mu[:, :])
            nc.vector.tensor_sub(t[:, :], t[:, :], musq[:, :])
            rsbf = sp1.tile([1, 512], BF16, tag="ln_rsbf")
            _act_raw(nc, rsbf[:, :], t[:, :], AF.Rsqrt, bias=epsr[:, :])
            mubf = sp1.tile([1, 512], BF16, tag="ln_mubf")
            _eng(nc).tensor_copy(mubf[:, :], mu[:, :])
            mu_b = psP.tile([128, 512], F32, tag="pp")
            nc.tensor.matmul(mu_b[:, :], ones_row[:, 0:128], mubf[:, :],
                             start=True, stop=True)
            rs_b = psP.tile([128, 512], F32, tag="pp")
            nc.tensor.matmul(rs_b[:, :], ones_row[:, 0:128], rsbf[:, :],
                             start=True, stop=True)
            qs = slice(qb * 512, qb * 512 + 512)
            g = bc[gname]
            be = bc[bename]
            for mt in range(MT):
                tmp = lnsc.tile([128, 512], F32, tag="lntmp")
                nc.vector.tensor_sub(tmp[:, :], z[:, mt, :], mu_b[:, :])
                nc.vector.tensor_mul(tmp[:, :], tmp[:, :], rs_b[:, :])
                nc.vector.tensor_scalar(
                    xout[:, mt, qs], tmp[:, :],
                    g[:, mt:mt + 1], be[:, mt:mt + 1],
                    op0=mybir.AluOpType.mult, op1=mybir.AluOpType.add,
                )
                if xf8out is not None:
                    nc.scalar.activation(xf8out[:, mt, qs], xout[:, mt, qs],
                                         AF.Copy, scale=SA)

        defer_q = []

        def flush_epilogues():
            for fn in defer_q:
                fn()
            defer_q.clear()

        def attention(KTh, VH, QTh, attn_out, masked, hp):
            """One head-pair of attention in transposed layout.
            KTh [128, S] bf16, VH [128, 8, 2, 2, 65] f8 (ktpair, kt, head),
            QTh [128, QL] bf16.  Causal masks are ADDED to the scores in
            PSUM (0 / -400) before exp; exp writes f8 so the attn-V matmuls
            run fp8 DoubleRow over k-tile pairs.  Writes attn_out[:, hp, :]
            (bf16), softmax-normalized.  The normalize epilogue is DEFERRED
            into the next slot's score stream so the PE never stalls."""
            for qb in range(2):
                nkt = SLOT_NKT[qb]
                qs = slice(qb * 512, qb * 512 + 512)
                us = []
                for _h in range(2):
                    u_t = psU.tile([65, 512], F32, tag="u")
                    us.append(u_t)
                for kt in range(nkt):
                    s2 = psS.tile([128, 2, 512], F32, tag="s2")
                    for h in range(2):
                        hs = slice(h * 64, h * 64 + 64)
                        nc.tensor.matmul(
                            s2[:, h, :],
                            KTh[hs, kt * 128:kt * 128 + 128],
                            QTh[hs, qs],
                            start=True, stop=True,
                        )
                    if kt == 2:
                        # previous slot's scores are in flight on the PE;
                        # emit the pending normalize epilogue now
                        flush_epilogues()
                    e2 = epool.tile([128, 2, 512], BF16, tag="e")
                    nc.scalar.activation(e2[:, :, :], s2[:, :, :], AF.Exp,
                                         scale=SCALE)
                    if masked and kt >= nkt - 8:
                        ki = kt - (nkt - 8)
                        off = (896 if qb == 0 else 1408) - 128 * ki
                        for h in range(2):
                            nc.vector.tensor_mul(e2[:, h, :], e2[:, h, :],
                                                 maskt[:, qb, off:off + 512])
                    for h in range(2):
                        nc.tensor.matmul(
                            us[h][:, :], VH[:, kt // 2, kt % 2, h, :, ],
                            e2[:, h, :],
                            start=(kt == 0), stop=(kt == nkt - 1),
                        )
                # normalize: move u + per-head sums to SBUF, one lane-parallel
                # reciprocal for both heads (rows 0 and 64), then defer the
                # broadcast+mul until the PE has new score work queued.
                # u rows 0-63 carry SV*sum(e*v); fold 1/SV into the sums.
                u_sbs = []
                sum2 = sp2.tile([65, 512], F32, tag="sum2")
                nc.vector.memset(sum2[:, :], 1.0)
                for h in range(2):
                    u_sb = sp2.tile([65, 512], F32, tag="u_sb", bufs=4)
                    nc.vector.tensor_copy(u_sb[:, :], us[h][:, :])
                    u_sbs.append(u_sb)
                nc.vector.tensor_scalar_mul(sum2[0:1, :],
                                             u_sbs[0][64:65, :], 1.0 / SAT)
                nc.vector.tensor_scalar_mul(sum2[64:65, :],
                                             u_sbs[1][64:65, :], 1.0 / SAT)
                rec2 = sp2.tile([65, 512], F32, tag="rec2")
                nc.vector.reciprocal(rec2[:, :], sum2[:, :])
                recbf = sp2.tile([65, 512], BF16, tag="recbf")
                nc.vector.tensor_copy(recbf[:, :], rec2[:, :])

                def epilogue(recbf=recbf, u_sbs=u_sbs, hp=hp, qs=qs):
                    for h in range(2):
                        hs = slice(h * 64, h * 64 + 64)
                        rb = psP.tile([128, 512], F32, tag="pp")
                        if h == 0:
                            nc.tensor.matmul(rb[0:64, :], ones_row[:, 0:64],
                                             recbf[0:1, :],
                                             start=True, stop=True)
                        else:
                            nc.tensor.matmul(rb[0:64, :], ones64[64:65, :],
                                             recbf[64:65, :],
                                             start=True, stop=True)
                        nc.vector.tensor_mul(
                            attn_out[hs, hp, qs], u_sbs[h][0:64, :], rb[0:64, :]
                        )

                defer_q.append(epilogue)

        # ================= PHASE A: self-attention =================
        pRES = open_pool(name="pRES", bufs=1, side="right")  # fp32 residual
        pAT = open_pool(name="pAT", bufs=1, side="right")    # attn1
        pV1 = open_pool(name="pV1", bufs=1, side="right")    # all-head V1

        xres_t = pRES.tile([128, MT, QL], F32, tag="xres_t")
        attn1 = pAT.tile([128, MT, QL], F8, tag="attn1")

        # ---- all-head V1 = SV*(x @ wv1 + bv1) in [tokens, d] f8 layout ----
        # moving = weight slices (512 wide), stationary = x token-tiles;
        # the bias lands in PSUM via a ones-row x rv1-row matmul.
        v1all = pV1.tile([128, 16, D], BF16, tag="v1all")
        for st in range(16):
            ts_ = slice(st * 128, st * 128 + 128)
            for db in range(2):
                dsl = slice(db * 512, db * 512 + 512)
                pp = psP.tile([128, 512], F32, tag="pp")
                for m2 in range(MT // 2):
                    nc.tensor.matmul(
                        pp[:, :],
                        xTs[:, 2 * m2:2 * m2 + 2, ts_],
                        wv1s[:, 2 * m2:2 * m2 + 2, dsl],
                        start=(m2 == 0), stop=False,
                        perf_mode=DR,
                    )
                nc.tensor.matmul(pp[:, :], ones_row[:, 0:128], rv1_sb[:, dsl],
                                 start=False, stop=True)
                nc.scalar.activation(v1all[:, st, dsl], pp[:, :], AF.Copy,
                                     scale=DS_XW)

        def make_A(hp):
            ds = slice(hp * 128, hp * 128 + 128)
            wq1s = wpool.tile([128, MT, 128], F8, tag="wq1s")
            wk1s = wpool.tile([128, MT, 128], F8, tag="wk1s")
            for nm, t in (("wq1", wq1s), ("wk1", wk1s)):
                nc.sync.dma_start(out=t, in_=wslices(nm)[:, :, ds])
            KTh = hpool.tile([128, S], BF16, tag="KTh")
            for sb in range(4):
                ss = slice(sb * 512, sb * 512 + 512)
                pp = psP.tile([128, 512], F32, tag="pp")
                for m2 in range(MT // 2):
                    nc.tensor.matmul(
                        pp[:, :],
                        wk1s[:, 2 * m2:2 * m2 + 2, :],
                        xTs[:, 2 * m2:2 * m2 + 2, ss],
                        start=(m2 == 0), stop=(m2 == MT // 2 - 1),
                        perf_mode=DR,
                    )
                nc.scalar.activation(KTh[:, ss], pp[:, :], AF.Identity,
                                     bias=bc["ck1"][:, hp:hp + 1], scale=DS_XW)
            QTh = hpool.tile([128, QL], BF16, tag="QTh")
            for qb in range(2):
                qs = slice(qb * 512, qb * 512 + 512)
                pp = psP.tile([128, 512], F32, tag="pp")
                for m2 in range(MT // 2):
                    nc.tensor.matmul(
                        pp[:, :],
                        wq1s[:, 2 * m2:2 * m2 + 2, :],
                        xqs[:, 2 * m2:2 * m2 + 2, qs],
                        start=(m2 == 0), stop=(m2 == MT // 2 - 1),
                        perf_mode=DR,
                    )
                nc.scalar.activation(QTh[:, qs], pp[:, :], AF.Identity,
                                     bias=bc["cq1"][:, hp:hp + 1], scale=DS_XW)
            VH = hpool.tile([128, 8, 2, 2, 65], BF16, tag="VH")
            nc.vector.memset(VH[:, :, :, :, 64:65], 1.0)
            for st in range(16):
                _eng(nc).tensor_copy(
                    VH[:, st // 2, st % 2, :, 0:64],
                    v1all[:, st, ds].rearrange("p (a b) -> p a b", a=2),
                )
            return KTh, QTh, VH

        preA = make_A(0)
        for hp in range(MT):
            curA = preA
            if hp + 1 < MT:
                preA = make_A(hp + 1)
            attention(curA[0], curA[2], curA[1], attn1, True, hp)
            if hp == 0:
                for mt in range(MT):
                    nc.sync.dma_start(
                        out=xres_t[:, mt, :],
                        in_=xres.ap().rearrange(
                            "(mt p) s -> p mt s", p=128)[:, mt, :],
                    )
        flush_epilogues()

        close_pool("pV1")   # free v1all
        close_pool("pA")  # free xT/xTq/mask

        # ---- enc loads + Q2 projection (independent of attn1) hoisted here
        # so their PE work fills the LN1/V2/AllGather serial window ----
        pENCS = open_pool(name="pENCS", bufs=1)
        pK2 = open_pool(name="pK2", bufs=1)
        pENCQ = open_pool(name="pENCQ", bufs=1)
        encs = pENCS.tile([128, MT, S], F8, tag="encs")
        for mt in range(MT):
            nc.sync.dma_start(
                out=encs[:, mt, :],
                in_=encT.ap().rearrange("(mt p) s -> p mt s", p=128)[:, mt, :],
            )
        encq = pENCQ.tile([128, MT, QL], F8, tag="encq")
        for mt in range(MT):
            nc.sync.dma_start(
                out=encq[:, mt, :],
                in_=encTq.ap().rearrange("(mt p) s -> p mt s", p=128)[:, mt, :],
            )
        Q2T = pK2.tile([128, MT, QL], BF16, tag="Q2T")
        for nt in range(MT):
            nsl = slice(nt * 128, nt * 128 + 128)
            wq2s = wpool.tile([128, MT, 128], F8, tag="wq1s")
            nc.sync.dma_start(out=wq2s, in_=wslices("wq2")[:, :, nsl])
            for qb in range(2):
                qs = slice(qb * 512, qb * 512 + 512)
                pp = psP.tile([128, 512], F32, tag="pp")
                for m2 in range(MT // 2):
                    nc.tensor.matmul(
                        pp[:, :],
                        wq2s[:, 2 * m2:2 * m2 + 2, :],
                        encq[:, 2 * m2:2 * m2 + 2, qs],
                        start=(m2 == 0), stop=(m2 == MT // 2 - 1),
                        perf_mode=DR,
                    )
                nc.scalar.activation(Q2T[:, nt, qs], pp[:, :], AF.Identity,
                                     bias=bc["cq2"][:, nt:nt + 1], scale=DS_XW)
        close_pool("pENCQ")

        # ---- out-proj 1 + residual -> z1, then LN1 -> x1 ----
        # weight tiles prefetched two (qb,nt)-steps ahead so the first
        # matmuls of each step never wait on DMA.
        def dma_wo(name, nt):
            w = wpool.tile([128, MT, 128], F8, tag="wo1s")
            nc.sync.dma_start(
                out=w, in_=wslices(name)[:, :, nt * 128:nt * 128 + 128])
            return w

        pZ = open_pool(name="pZ", bufs=1)
        z1 = pZ.tile([128, MT, QL], F32, tag="z1")
        steps = [(qb, nt) for qb in range(2) for nt in range(MT)]
        wo_pre = [dma_wo("wo1", steps[0][1]), dma_wo("wo1", steps[1][1])]
        for i, (qb, nt) in enumerate(steps):
            qs = slice(qb * 512, qb * 512 + 512)
            wo1s = wo_pre[i % 2]
            pp = psP.tile([128, 512], F32, tag="pp")
            for d2 in range(MT // 2):
                nc.tensor.matmul(pp[:, :], wo1s[:, 2 * d2:2 * d2 + 2, :],
                                 attn1[:, 2 * d2:2 * d2 + 2, qs],
                                 start=(d2 == 0), stop=False, perf_mode=DR)
            nc.tensor.matmul(pp[:, :], ro1_sb[:, nt * 128:nt * 128 + 128],
                             ones_row[:, :], start=False, stop=True)
            nc.vector.scalar_tensor_tensor(
                z1[:, nt, qs], pp[:, :], DS_OW,
                xres_t[:, nt, qs], op0=MUL, op1=ADD,
            )
            if i + 2 < len(steps):
                wo_pre[i % 2] = dma_wo("wo1", steps[i + 2][1])

        close_pool("pAT")   # free attn1 (right stack top)
        close_pool("pRES")  # free xres

        pX1 = open_pool(name="pX1", bufs=1)
        pX1B = open_pool(name="pX1B", bufs=1)
        x1 = pX1.tile([128, MT, QL], F32, tag="x1")
        x1f8 = pX1B.tile([128, MT, QL], F8, tag="x1f8")
        for qb in range(2):
            qs = slice(qb * 512, qb * 512 + 512)
            ln_block(z1[:, :, qs], "g1", "be1", x1, x1f8, qb)

        # ================= V2 projection + split AllGather =================
        pV2 = open_pool(name="pV2", bufs=1)
        wv2s = wpool.tile([128, MT, D], F8, tag="wvfull", bufs=1)
        nc.sync.dma_start(out=wv2s, in_=wslices("wv2"))
        v2sb = pV2.tile([128, MT, D], BF16, tag="v2sb")
        for half, (v2loc_h, v2all_h) in enumerate(
                ((v2locA, v2allA), (v2locB, v2allB))):
            for st in range(4 * half, 4 * half + 4):
                ss = slice(st * 128, st * 128 + 128)
                for db in range(2):
                    dsl = slice(db * 512, db * 512 + 512)
                    pp = psP.tile([128, 512], F32, tag="pp")
                    for m2 in range(MT // 2):
                        nc.tensor.matmul(
                            pp[:, :],
                            x1f8[:, 2 * m2:2 * m2 + 2, ss],
                            wv2s[:, 2 * m2:2 * m2 + 2, dsl],
                            start=(m2 == 0), stop=False,
                            perf_mode=DR,
                        )
                    nc.tensor.matmul(pp[:, :], ones_row[:, 0:128],
                                     rv2_sb[:, dsl], start=False, stop=True)
                    nc.scalar.activation(v2sb[:, st, dsl], pp[:, :],
                                         AF.Copy, scale=DS_AW)
            nc.sync.dma_start(
                out=v2loc_h.ap().rearrange("(st p) d -> p st d", p=128),
                in_=v2sb[:, 4 * half:4 * half + 4, :],
            )
            nc.gpsimd.collective_compute(
                "AllGather",
                mybir.AluOpType.bypass,
                replica_groups=[[2 * p, 2 * p + 1] for p in range(4)],
                ins=[v2loc_h[:, :]],
                outs=[v2all_h[:, :]],
            )
        close_pool("pV2")
        close_pool("pX1B")  # x1f8 only needed for the V2 projection

        # ================= PHASE B: cross-attention =================
        pAT2 = open_pool(name="pAT2", bufs=1)

        attn2 = pAT2.tile([128, MT, QL], F8, tag="attn2")

        def make_B(hp):
            ds = slice(hp * 128, hp * 128 + 128)
            wk2s = wpool.tile([128, MT, 128], F8, tag="wk1s")
            nc.sync.dma_start(out=wk2s, in_=wslices("wk2")[:, :, ds])
            K2h = hpool.tile([128, S], BF16, tag="KTh")
            for sb in range(4):
                ss = slice(sb * 512, sb * 512 + 512)
                pp = psP.tile([128, 512], F32, tag="pp")
                for m2 in range(MT // 2):
                    nc.tensor.matmul(
                        pp[:, :],
                        wk2s[:, 2 * m2:2 * m2 + 2, :],
                        encs[:, 2 * m2:2 * m2 + 2, ss],
                        start=(m2 == 0), stop=(m2 == MT // 2 - 1),
                        perf_mode=DR,
                    )
                nc.scalar.activation(K2h[:, ss], pp[:, :], AF.Identity,
                                     bias=bc["ck2"][:, hp:hp + 1], scale=DS_XW)
            VH2 = hpool.tile([128, 8, 2, 2, 65], BF16, tag="VH")
            nc.vector.memset(VH2[:, :, :, :, 64:65], 1.0)
            for t in range(16):
                if t < 8:
                    vsrc, row0 = v2allA, t * 128
                elif t < 12:
                    vsrc, row0 = v2allB, (t - 4) * 128
                else:
                    vsrc, row0 = v2allB, (t - 12) * 128
                nc.sync.dma_start(
                    out=VH2[:, t // 2, t % 2, :, 0:64],
                    in_=vsrc[row0:row0 + 128,
                             hp * 128:hp * 128 + 128].rearrange(
                        "p (a b) -> p a b", a=2),
                )
            return K2h, VH2

        preB = make_B(0)
        for hp in range(MT):
            curB = preB
            if hp + 1 < MT:
                preB = make_B(hp + 1)
            attention(curB[0], curB[1], Q2T[:, hp, :], attn2, False, hp)

        # ---- out-proj 2 + residual -> z2, then LN2 -> x2 ----
        z2 = pZ.tile([128, MT, QL], F32, tag="z1")
        wo_pre = [dma_wo("wo2", steps[0][1]), dma_wo("wo2", steps[1][1])]
        for i, (qb, nt) in enumerate(steps):
            qs = slice(qb * 512, qb * 512 + 512)
            if i == 2:
                flush_epilogues()
            wo2s = wo_pre[i % 2]
            pp = psP.tile([128, 512], F32, tag="pp")
            for d2 in range(MT // 2):
                nc.tensor.matmul(pp[:, :], wo2s[:, 2 * d2:2 * d2 + 2, :],
                                 attn2[:, 2 * d2:2 * d2 + 2, qs],
                                 start=(d2 == 0), stop=False, perf_mode=DR)
            nc.tensor.matmul(pp[:, :], ro2_sb[:, nt * 128:nt * 128 + 128],
                             ones_row[:, :], start=False, stop=True)
            nc.vector.scalar_tensor_tensor(
                z2[:, nt, qs], pp[:, :], DS_OW,
                x1[:, nt, qs], op0=MUL, op1=ADD,
            )
            if i + 2 < len(steps):
                wo_pre[i % 2] = dma_wo("wo2", steps[i + 2][1])

        close_pool("pAT2")
        close_pool("pX1")

        pX2 = open_pool(name="pX2", bufs=1, side="right")
        pX2B = open_pool(name="pX2B", bufs=1, side="right")
        x2 = pX2.tile([128, MT, QL], F32, tag="x2")
        x2f8 = pX2B.tile([128, MT, QL], F8, tag="x2f8")
        for qb in range(2):
            qs = slice(qb * 512, qb * 512 + 512)
            ln_block(z2[:, :, qs], "g2", "be2", x2, x2f8, qb)
        close_pool("pZ")
        close_pool("pK2")
        close_pool("pENCS")

        # ================= PHASE C: FFN + LN3 -> out =================
        pF = open_pool(name="pF", bufs=1, side="right")
        pF2 = open_pool(name="pF2", bufs=2)
        hT = pF.tile([128, FT, QL], F8, tag="hT")

        def dma_wf1(ft):
            wf1s = pF2.tile([128, MT, 128], F8, tag="wf1s")
            nc.sync.dma_start(
                out=wf1s,
                in_=wd["wf1"].ap().rearrange("(mt p) f -> p mt f", p=128)[
                    :, :, ft * 128:ft * 128 + 128],
            )
            return wf1s

        wf1_pre = dma_wf1(0)
        for ft in range(FT):
            wf1s = wf1_pre
            if ft + 1 < FT:
                wf1_pre = dma_wf1(ft + 1)
            for qb in range(2):
                qs = slice(qb * 512, qb * 512 + 512)
                pp = psP.tile([128, 512], F32, tag="pp")
                for m2 in range(MT // 2):
                    nc.tensor.matmul(
                        pp[:, :],
                        wf1s[:, 2 * m2:2 * m2 + 2, :],
                        x2f8[:, 2 * m2:2 * m2 + 2, qs],
                        start=(m2 == 0), stop=(m2 == MT // 2 - 1),
                        perf_mode=DR,
                    )
                # hT = SA * relu(psum*DS_AW + bf1); cf1 is host-scaled by SA
                nc.scalar.activation(hT[:, ft, qs], pp[:, :], AF.Relu,
                                     bias=bc["cf1"][:, ft:ft + 1],
                                     scale=DS_AW * SA)
        # z3 overwrites x2 in place: x2's last use is this residual add.
        # qb-outer so LN3+store of qb0 overlap FF2 matmuls of qb1.
        def dma_wf2(nt):
            wf2s = pF2.tile([128, FT, 128], F8, tag="wf2s")
            nc.sync.dma_start(
                out=wf2s,
                in_=wd["wf2"].ap().rearrange("(ft p) d -> p ft d", p=128)[
                    :, :, nt * 128:nt * 128 + 128],
            )
            return wf2s

        wf2_pre = dma_wf2(0)
        for qb in range(2):
            qs = slice(qb * 512, qb * 512 + 512)
            for nt in range(MT):
                wf2s = wf2_pre
                if nt + 1 < MT or qb == 0:
                    wf2_pre = dma_wf2((nt + 1) % MT)
                pp = psP.tile([128, 512], F32, tag="pp")
                for f2 in range(FT // 2):
                    nc.tensor.matmul(
                        pp[:, :],
                        wf2s[:, 2 * f2:2 * f2 + 2, :],
                        hT[:, 2 * f2:2 * f2 + 2, qs],
                        start=(f2 == 0), stop=(f2 == FT // 2 - 1),
                        perf_mode=DR,
                    )
                t1 = lnsc.tile([128, 512], F32, tag="lntmp")
                nc.scalar.activation(t1[:, :], pp[:, :], AF.Identity,
                                     bias=bc["cf2"][:, nt:nt + 1], scale=DS_H2)
                nc.vector.tensor_add(x2[:, nt, qs], t1[:, :], x2[:, nt, qs])
            outsb = pF.tile([128, MT, 512], F32, tag="outsb")
            ln_block(x2[:, :, qs], "g3", "be3", outsb, None, 0)
            nc.sync.dma_start(
                out=outT.ap().rearrange("(mt p) q -> p mt q", p=128)[:, :, qs],
                in_=outsb,
            )
        close_pool("pF2")
        close_pool("pF")
        close_pool("pX2B")
        close_pool("pX2")

        for nm in reversed(list(_cms)):
            close_pool(nm)

    return nc


_CACHED = {}


def _get_nc():
    if "nc" not in _CACHED:
        nc = build_nc()
        legalize_waits(nc)
        _CACHED["nc"] = nc
    return _CACHED["nc"]


def _colbias(v, k=8):
    return np.ascontiguousarray(np.asarray(v, np.float32).reshape(k, 128).T)


def _bf(a):
    return np.ascontiguousarray(np.asarray(a)).astype(ml_dtypes.bfloat16)


def _f8(a, scale):
    a = np.asarray(a, np.float32) * scale
    return np.clip(a, -224.0, 224.0).astype(ml_dtypes.float8_e4m3)


def _make_mask(j):
    # ramp[sl][i, u] = (u >= i + c) with c = (896, 896) for j=0 and
    # (384, 1408) for j=1; windows at (896,1408)[sl] - 128*ki reproduce
    # the per-(slot, k-tile) causal masks.
    cs = (896, 896) if j == 0 else (384, 1408)
    i = np.arange(128)[:, None]
    u = np.arange(1920)[None, :]
    m = np.stack([(u >= i + c).astype(np.float32) for c in cs])
    return m.astype(ml_dtypes.bfloat16)


def kernel(**inputs):
    x = np.asarray(inputs["x"], np.float32)
    enc = np.asarray(inputs["encoder_output"], np.float32)
    shared = {}
    for name in ("wq1", "wk1", "wv1", "wq2", "wk2", "wv2"):
        shared[name] = _f8(inputs[name], SW)
    shared["wf1"] = _f8(inputs["wf1"], SW)
    shared["wf2"] = _f8(inputs["wf2"], SW2)
    for name in ("wo1", "wo2"):
        shared[name] = _f8(inputs[name], SWO)
    shared["ro1"] = _bf(np.asarray(inputs["bo1"], np.float32).reshape(1, D)
                        * (SAT * SWO))
    shared["ro2"] = _bf(np.asarray(inputs["bo2"], np.float32).reshape(1, D)
                        * (SAT * SWO))
    for src, dst in (("bq1", "cq1"), ("bk1", "ck1"), ("bo1", "co1"),
                     ("bq2", "cq2"), ("bk2", "ck2"), ("bo2", "co2"),
                     ("g1", "g1"), ("be1", "be1"), ("g2", "g2"), ("be2", "be2"),
                     ("g3", "g3"), ("be3", "be3")):
        shared[dst] = _colbias(inputs[src], 8)
    shared["cf1"] = _colbias(np.asarray(inputs["bf1"], np.float32) * SA, 32)
    shared["cf2"] = _colbias(inputs["bf2"], 8)
    shared["rv1"] = _bf(np.asarray(inputs["bv1"], np.float32).reshape(1, D)
                        * (SX * SW))
    shared["rv2"] = _bf(np.asarray(inputs["bv2"], np.float32).reshape(1, D)
                        * (SA * SW))
    masks = {0: _make_mask(0), 1: _make_mask(1)}

    in_maps = []
    col_list = []
    for c in range(NCORES):
        b, j = c // 2, c % 2
        q0a, q0b = (0, 1536) if j == 0 else (512, 1024)
        cols = np.r_[q0a:q0a + 512, q0b:q0b + 512]
        col_list.append((b, cols))
        xTb = np.ascontiguousarray(x[b].T)
        encTb = np.ascontiguousarray(enc[b].T)
        m = dict(shared)
        m["xT"] = _f8(xTb, SX)
        m["xTq"] = _f8(xTb[:, cols], SX)
        m["xres"] = np.ascontiguousarray(xTb[:, cols])
        m["encT"] = _f8(encTb, SX)
        m["encTq"] = _f8(encTb[:, cols], SX)
        m["bigmask"] = masks[j]
        in_maps.append(m)

    global _LAST_IN_MAPS
    _LAST_IN_MAPS = in_maps
    nc = _get_nc()
    res = run_bass_kernel_spmd(nc, in_maps, core_ids=list(range(NCORES)))
    out = np.empty((B, S, D), np.float32)
    for c in range(NCORES):
        b, cols = col_list[c]
        out[b, cols, :] = res.results[c]["outT"].T
    return out


# revision 92
# speedup vs baseline: 1.0053x; 1.0053x over previous
"""Trainium2 Bass kernel for a post-LN transformer decoder layer.

Sharding: 8 cores = 4 batches x 2 token-halves. Core c=(b,j) handles batch b
and 2 query quarter-blocks (j=0: global rows [0:512)+[1536:2048),
j=1: [512:1536)) -- interleaved quarters balance the causal triangle.

Key optimizations over the naive pipeline:
- All projections (Q/K/V both attns) and the FFN run as fp8e4 DoubleRow
  matmuls (0.5 cycles/row): host quantizes x/enc/weights to e4m3 with
  power-of-2 scales; descales fold into the existing bias activations.
  Attention scores / attn-V matmuls stay bf16 (an all-fp8 attention was
  tried and made the whole PE clock-throttle harder -- net loss).
- V for all heads is computed upfront in [tokens, d] layout with 512-wide
  moving dims (2.25x fewer PE rows than per-head-pair V); V biases land in
  PSUM via a ones-row matmul so the SBUF copy is a pure scaled ACT copy.
- Exp is fused across each head pair (one ACT op per k-tile over a 2-bank
  PSUM tile); softmax reciprocals run as ONE lane-parallel DVE reciprocal
  per head-pair (rows 0/64), and the normalize epilogue is deferred into
  the next slot's score stream so the PE never stalls on it.
- Per-head-pair inputs (weight DMAs, K/Q projections, V copies) are
  emitted one head-pair ahead; enc DMAs + the Q2 projection are hoisted
  before LN1 so their PE work fills the LN1/V2/AllGather serial window;
  FFN weights prefetch one tile ahead and are loaded once.
- LN mean/var run via ones-column matmuls on bf16 copies (GpSimd casts),
  rstd uses a raw ACT Rsqrt (accuracy ample for LN), out-proj bias +
  residual add fuse into one DVE scalar_tensor_tensor.
- The V2 = x1 @ wv2 AllGather (the only collective) is split in two
  halves so cross-attention can consume the first half early.

Residual/LN path is fp32.  The program must be identical on all 8 cores
(single SPMD NEFF), so the causal structure is padded to a uniform (8, 16)
k-tile schedule per query slot and causal masking multiplies exp outputs
by per-core 0/1 ramp windows (two shared [128,1920] ramps, static offsets).
"""

import sys

sys.path.insert(0, "/opt/trn_rl_repo")

import numpy as np
import ml_dtypes

import os

import concourse.bass as bass
import concourse.tile as tile
from concourse import mybir
from concourse.bass_utils import run_bass_kernel_spmd

USE_FAST_RECIP = os.environ.get("K_FAST_RECIP", "0") == "1"
USE_GPSIMD = os.environ.get("K_GPSIMD", "1") == "1"

BF16 = mybir.dt.bfloat16
F8 = mybir.dt.float8e4
F32 = mybir.dt.float32
AF = mybir.ActivationFunctionType
DR = mybir.MatmulPerfMode.DoubleRow
MUL = mybir.AluOpType.mult
ADD = mybir.AluOpType.add

D = 1024       # d_model
DFF = 4096
B, S = 4, 2048
NCORES = 8
QL = 1024      # local query rows per core
EPS = 1e-6
MT = 8         # d_model 128-tiles
FT = 32        # d_ff 128-tiles
SCALE = 0.125  # 1/sqrt(head_dim)
SLOT_NKT = (8, 16)  # uniform k-tile count per query slot; last 8 are masked

SX = 32.0      # host fp8 scale for x / enc
SW = 4096.0    # host fp8 scale for d_model-input weights
SW2 = 8192.0   # host fp8 scale for wf2 (d_ff input)
SA = 16.0      # on-chip fp8 scale for LN outputs (x1, x2) and ffn hidden
SV = 16.0      # on-chip fp8 scale for V tiles (attn values)
SAT = 32.0     # on-chip fp8 scale for normalized attention outputs
SWO = 4096.0   # host fp8 scale for out-proj weights
DS_OW = 1.0 / (SAT * SWO)  # descale for attn @ wo
DS_XW = 1.0 / (SX * SW)    # descale for x/enc @ w projections
DS_AW = 1.0 / (SA * SW)    # descale for x1/x2 @ w projections
DS_H2 = 1.0 / (SA * SW2)   # descale for h @ wf2
MASK_NEG = -400.0          # additive mask on scores; exp(-50) == 0 in f32

# AllGather is per core-pair; pair-local rank j owns quarters (Q0,Q3) for j=0
# and (Q1,Q2) for j=1.  v2all row base for global k-tile t is
# V2_ROW0[t//4] + (t%4)*128.
V2_ROW0 = [0, 1024, 1536, 512]


def _eng(nc):
    return nc.gpsimd if USE_GPSIMD else nc.vector


def _act_raw(nc, out, in_, func, bias=0.0, scale=1.0):
    """Raw InstActivation emission: out = func(in_*scale + bias).

    Bypasses bass's Reciprocal/Rsqrt accuracy guard -- the attention
    normalizer and LN rstd only need ~1% relative accuracy, and the ACT
    LUT versions are ~5x faster than DVE's microcoded reciprocal."""
    eng = nc.scalar
    inputs = [eng.lower_ap(in_)]
    for arg in (bias, scale, 0.0):
        if isinstance(arg, bass.AP):
            inputs.append(eng.lower_ap(arg))
        else:
            inputs.append(mybir.ImmediateValue(dtype=mybir.dt.float32, value=arg))
    return eng.add_instruction(
        mybir.InstActivation(
            name=nc.get_next_instruction_name(),
            func=func,
            ins=inputs,
            outs=[eng.lower_ap(out)],
        )
    )


def legalize_waits(nc, max_waits=1):
    """This walrus build accepts at most one sync-wait per instruction.
    Hoist excess waits onto same-engine NoOps inserted just before."""
    nid = 0
    for fn in nc.m.functions:
        for bb in fn.blocks:
            new = []
            changed = False
            for inst in bb.instructions:
                si = inst.sync_info
                if si is not None and si.on_wait and len(si.on_wait) > max_waits:
                    waits = list(si.on_wait)
                    for w in waits[:-max_waits]:
                        nid += 1
                        nop = mybir.InstNoOp(name=f"I-waitfix-{nid}", ins=[], outs=[])
                        nop.engine = inst.engine
                        nop.sync_info = mybir.SyncInfo(on_wait=[w], on_update=[])
                        new.append(nop)
                    inst.sync_info = mybir.SyncInfo(
                        on_wait=waits[-max_waits:], on_update=list(si.on_update)
                    )
                    changed = True
                new.append(inst)
            if changed:
                bb.instructions = new


def build_nc():
    nc = bass.Bass(num_devices=NCORES)

    xT = nc.dram_tensor("xT", [D, S], F8, kind="ExternalInput")
    xTq = nc.dram_tensor("xTq", [D, QL], F8, kind="ExternalInput")
    xres = nc.dram_tensor("xres", [D, QL], F32, kind="ExternalInput")
    encT = nc.dram_tensor("encT", [D, S], F8, kind="ExternalInput")
    encTq = nc.dram_tensor("encTq", [D, QL], F8, kind="ExternalInput")
    wd = {}
    for name in ("wq1", "wk1", "wv1", "wq2", "wk2", "wv2"):
        wd[name] = nc.dram_tensor(name, [D, D], F8, kind="ExternalInput")
    for name in ("wo1", "wo2"):
        wd[name] = nc.dram_tensor(name, [D, D], F8, kind="ExternalInput")
    # out-proj bias rows, host-prescaled by SAT*SWO (added in PSUM via a
    # stationary-row x ones matmul, like the V biases)
    ro1 = nc.dram_tensor("ro1", [1, D], BF16, kind="ExternalInput")
    ro2 = nc.dram_tensor("ro2", [1, D], BF16, kind="ExternalInput")
    rf2 = nc.dram_tensor("rf2", [1, D], BF16, kind="ExternalInput")
    wd["wf1"] = nc.dram_tensor("wf1", [D, DFF], F8, kind="ExternalInput")
    wd["wf2"] = nc.dram_tensor("wf2", [DFF, D], F8, kind="ExternalInput")
    bias_specs = (
        ("cq1", 8), ("ck1", 8), ("co1", 8), ("cq2", 8), ("ck2", 8), ("co2", 8),
        ("cf1", 32), ("cf2", 8),
        ("g1", 8), ("be1", 8), ("g2", 8), ("be2", 8), ("g3", 8), ("be3", 8),
    )
    bcd = {}
    for name, k in bias_specs:
        bcd[name] = nc.dram_tensor(name, [128, k], F32, kind="ExternalInput")
    # V bias rows, host-prescaled to the V psum scale (SX*SW / SA*SW)
    rv1 = nc.dram_tensor("rv1", [1, D], BF16, kind="ExternalInput")
    rv2 = nc.dram_tensor("rv2", [1, D], BF16, kind="ExternalInput")
    # Causal masks for (slot, k-tile) are shifted windows of two per-slot
    # ramps: ramp[sl][i, u] = (u >= i + c(core, sl)); window offset for
    # (sl, ki) is (896, 1408)[sl] - 128*ki.
    bigmask = nc.dram_tensor("bigmask", [2, 128, 1920], BF16, kind="ExternalInput")
    outT = nc.dram_tensor("outT", [D, QL], F32, kind="ExternalOutput")
    # V2 AllGather is split in two halves so cross-attention can start on
    # the first half while the second is still in flight.  Half A carries
    # each core's qb0 token tiles (global tiles 0-7 in order), half B the
    # qb1 tiles (global 12-15 then 8-11).
    v2locA = nc.dram_tensor("v2locA", [QL // 2, D], BF16)
    v2locB = nc.dram_tensor("v2locB", [QL // 2, D], BF16)
    v2allA = nc.dram_tensor("v2allA", [QL, D], BF16)
    v2allB = nc.dram_tensor("v2allB", [QL, D], BF16)

    def wslices(name):
        return wd[name].ap().rearrange("(mt p) d -> p mt d", p=128)

    with tile.TileContext(nc) as tc:
        _cms = {}

        def open_pool(**kw):
            cm = tc.tile_pool(**kw)
            _cms[kw["name"]] = cm
            return cm.__enter__()

        def close_pool(pool_name):
            _cms.pop(pool_name).__exit__(None, None, None)

        # long-lived pools (whole kernel)
        const = open_pool(name="const", bufs=1)
        wpool = open_pool(name="wpool", bufs=2)
        hpool = open_pool(name="hpool", bufs=2)
        epool = open_pool(name="epool", bufs=4)
        sp1 = open_pool(name="sp1", bufs=1)    # LN row scratch
        sp2 = open_pool(name="sp2", bufs=2)    # attention normalize scratch
        lnsc = open_pool(name="lnsc", bufs=2)  # per-mt LN scratch
        # PSUM: s2 tag = [128,2,512] (2 banks) x2 bufs = 4 banks;
        # u tag = 1 bank x2; pp tag = 1 bank x2  -> exactly 8 banks.
        psS = open_pool(name="psS", bufs=2, space="PSUM")
        psU = open_pool(name="psU", bufs=2, space="PSUM")
        psP = open_pool(name="psP", bufs=2, space="PSUM")

        # ---- phase-A inputs first: the first PE work (V1) needs these,
        # and two dozen tiny bias DMAs would otherwise delay it ----
        pA = open_pool(name="pA", bufs=1)                    # xT/xTq/mask
        xTs = pA.tile([128, MT, S], F8, tag="xTs")
        for mt in range(MT):
            nc.sync.dma_start(
                out=xTs[:, mt, :],
                in_=xT.ap().rearrange("(mt p) s -> p mt s", p=128)[:, mt, :],
            )
        wv1s = wpool.tile([128, MT, D], F8, tag="wvfull", bufs=1)
        nc.sync.dma_start(out=wv1s, in_=wslices("wv1"))
        xqs = pA.tile([128, MT, QL], F8, tag="xqs")
        for mt in range(MT):
            nc.sync.dma_start(
                out=xqs[:, mt, :],
                in_=xTq.ap().rearrange("(mt p) s -> p mt s", p=128)[:, mt, :],
            )
        maskt = pA.tile([128, 2, 1920], BF16, tag="maskt")
        nc.sync.dma_start(
            out=maskt, in_=bigmask.ap().rearrange("sl p u -> p sl u")
        )

        # ---- constants ----
        bc = {}
        for name, k in bias_specs:
            t = const.tile([128, k], F32, tag=name)
            nc.sync.dma_start(out=t, in_=bcd[name][:, :])
            bc[name] = t
        rv1_sb = const.tile([1, D], BF16, tag="rv1")
        nc.sync.dma_start(out=rv1_sb, in_=rv1[:, :])
        rv2_sb = const.tile([1, D], BF16, tag="rv2")
        nc.sync.dma_start(out=rv2_sb, in_=rv2[:, :])
        ro1_sb = const.tile([1, D], BF16, tag="ro1")
        nc.sync.dma_start(out=ro1_sb, in_=ro1[:, :])
        ro2_sb = const.tile([1, D], BF16, tag="ro2")
        nc.sync.dma_start(out=ro2_sb, in_=ro2[:, :])
        rf2_sb = const.tile([1, D], BF16, tag="rf2")
        nc.sync.dma_start(out=rf2_sb, in_=rf2[:, :])
        ones_row = const.tile([1, 512], BF16, tag="ones_row")
        nc.vector.memset(ones_row, 1.0)
        # ones row living at partition 64: stationary for the h=1 normalize
        # broadcast (matmul requires stationary/moving partition bases match)
        ones64 = const.tile([65, 64], BF16, tag="ones64")
        nc.vector.memset(ones64[64:65, :], 1.0)
        ones_col = const.tile([128, 1], BF16, tag="ones_col")
        nc.vector.memset(ones_col, 1.0)
        epsr = const.tile([1, 1], F32, tag="epsr")
        nc.vector.memset(epsr, EPS)

        def ln_block(z, gname, bename, xout, xf8out, qb):
            """LayerNorm over the partition(d_model) axis of z [128, MT, 512]
            f32.  Writes xout[:, :, qb*512:+512] f32 (+ optional f8 copy
            scaled by SA)."""
            psum_s = psU.tile([65, 512], F32, tag="u")
            psum_q = psU.tile([65, 512], F32, tag="u")
            for mt in range(MT):
                zbf = lnsc.tile([128, 512], BF16, tag="zbf")
                _eng(nc).tensor_copy(zbf[:, :], z[:, mt, :])
                zsq = lnsc.tile([128, 512], BF16, tag="zsq")
                _eng(nc).tensor_mul(zsq[:, :], zbf[:, :], zbf[:, :])
                nc.tensor.matmul(psum_s[0:1, :], ones_col[:, :], zbf[:, :],
                                 start=(mt == 0), stop=(mt == MT - 1))
                nc.tensor.matmul(psum_q[0:1, :], ones_col[:, :], zsq[:, :],
                                 start=(mt == 0), stop=(mt == MT - 1))
            mu = sp1.tile([1, 512], F32, tag="ln_mu")
            nc.scalar.activation(mu[:, :], psum_s[0:1, :], AF.Copy, scale=1.0 / D)
            t = sp1.tile([1, 512], F32, tag="ln_t")
            nc.scalar.activation(t[:, :], psum_q[0:1, :], AF.Copy, scale=1.0 / D)
            musq = sp1.tile([1, 512], F32, tag="ln_musq")
            nc.vector.tensor_mul(musq[:, :], mu[:, :], mu[:, :])
            nc.vector.tensor_sub(t[:, :], t[:, :], musq[:, :])
            rsbf = sp1.tile([1, 512], BF16, tag="ln_rsbf")
            _act_raw(nc, rsbf[:, :], t[:, :], AF.Rsqrt, bias=epsr[:, :])
            mubf = sp1.tile([1, 512], BF16, tag="ln_mubf")
            _eng(nc).tensor_copy(mubf[:, :], mu[:, :])
            mu_b = psP.tile([128, 512], F32, tag="pp")
            nc.tensor.matmul(mu_b[:, :], ones_row[:, 0:128], mubf[:, :],
                             start=True, stop=True)
            rs_b = psP.tile([128, 512], F32, tag="pp")
            nc.tensor.matmul(rs_b[:, :], ones_row[:, 0:128], rsbf[:, :],
                             start=True, stop=True)
            qs = slice(qb * 512, qb * 512 + 512)
            g = bc[gname]
            be = bc[bename]
            for mt in range(MT):
                tmp = lnsc.tile([128, 512], F32, tag="lntmp")
                nc.vector.tensor_sub(tmp[:, :], z[:, mt, :], mu_b[:, :])
                nc.vector.tensor_mul(tmp[:, :], tmp[:, :], rs_b[:, :])
                nc.vector.tensor_scalar(
                    xout[:, mt, qs], tmp[:, :],
                    g[:, mt:mt + 1], be[:, mt:mt + 1],
                    op0=mybir.AluOpType.mult, op1=mybir.AluOpType.add,
                )
                if xf8out is not None:
                    nc.scalar.activation(xf8out[:, mt, qs], xout[:, mt, qs],
                                         AF.Copy, scale=SA)

        defer_q = []

        def flush_epilogues():
            for fn in defer_q:
                fn()
            defer_q.clear()

        def attention(KTh, VH, QTh, attn_out, masked, hp):
            """One head-pair of attention in transposed layout.
            KTh [128, S] bf16, VH [128, 8, 2, 2, 65] f8 (ktpair, kt, head),
            QTh [128, QL] bf16.  Causal masks are ADDED to the scores in
            PSUM (0 / -400) before exp; exp writes f8 so the attn-V matmuls
            run fp8 DoubleRow over k-tile pairs.  Writes attn_out[:, hp, :]
            (bf16), softmax-normalized.  The normalize epilogue is DEFERRED
            into the next slot's score stream so the PE never stalls."""
            for qb in range(2):
                nkt = SLOT_NKT[qb]
                qs = slice(qb * 512, qb * 512 + 512)
                us = []
                for _h in range(2):
                    u_t = psU.tile([65, 512], F32, tag="u")
                    us.append(u_t)
                for kt in range(nkt):
                    s2 = psS.tile([128, 2, 512], F32, tag="s2")
                    for h in range(2):
                        hs = slice(h * 64, h * 64 + 64)
                        nc.tensor.matmul(
                            s2[:, h, :],
                            KTh[hs, kt * 128:kt * 128 + 128],
                            QTh[hs, qs],
                            start=True, stop=True,
                        )
                    if kt == 2:
                        # previous slot's scores are in flight on the PE;
                        # emit the pending normalize epilogue now
                        flush_epilogues()
                    e2 = epool.tile([128, 2, 512], BF16, tag="e")
                    nc.scalar.activation(e2[:, :, :], s2[:, :, :], AF.Exp,
                                         scale=SCALE)
                    if masked and kt >= nkt - 8:
                        ki = kt - (nkt - 8)
                        off = (896 if qb == 0 else 1408) - 128 * ki
                        for h in range(2):
                            nc.vector.tensor_mul(e2[:, h, :], e2[:, h, :],
                                                 maskt[:, qb, off:off + 512])
                    for h in range(2):
                        nc.tensor.matmul(
                            us[h][:, :], VH[:, kt // 2, kt % 2, h, :, ],
                            e2[:, h, :],
                            start=(kt == 0), stop=(kt == nkt - 1),
                        )
                # normalize: move u + per-head sums to SBUF, one lane-parallel
                # reciprocal for both heads (rows 0 and 64), then defer the
                # broadcast+mul until the PE has new score work queued.
                # u rows 0-63 carry SV*sum(e*v); fold 1/SV into the sums.
                u_sbs = []
                sum2 = sp2.tile([65, 512], F32, tag="sum2")
                nc.vector.memset(sum2[:, :], 1.0)
                for h in range(2):
                    u_sb = sp2.tile([65, 512], F32, tag="u_sb", bufs=4)
                    nc.vector.tensor_copy(u_sb[:, :], us[h][:, :])
                    u_sbs.append(u_sb)
                nc.vector.tensor_scalar_mul(sum2[0:1, :],
                                             u_sbs[0][64:65, :], 1.0 / SAT)
                nc.vector.tensor_scalar_mul(sum2[64:65, :],
                                             u_sbs[1][64:65, :], 1.0 / SAT)
                rec2 = sp2.tile([65, 512], F32, tag="rec2")
                nc.vector.reciprocal(rec2[:, :], sum2[:, :])
                recbf = sp2.tile([65, 512], BF16, tag="recbf")
                nc.vector.tensor_copy(recbf[:, :], rec2[:, :])

                def epilogue(recbf=recbf, u_sbs=u_sbs, hp=hp, qs=qs):
                    for h in range(2):
                        hs = slice(h * 64, h * 64 + 64)
                        rb = psP.tile([128, 512], F32, tag="pp")
                        if h == 0:
                            nc.tensor.matmul(rb[0:64, :], ones_row[:, 0:64],
                                             recbf[0:1, :],
                                             start=True, stop=True)
                        else:
                            nc.tensor.matmul(rb[0:64, :], ones64[64:65, :],
                                             recbf[64:65, :],
                                             start=True, stop=True)
                        nc.vector.tensor_mul(
                            attn_out[hs, hp, qs], u_sbs[h][0:64, :], rb[0:64, :]
                        )

                defer_q.append(epilogue)

        # ================= PHASE A: self-attention =================
        pRES = open_pool(name="pRES", bufs=1, side="right")  # fp32 residual
        pAT = open_pool(name="pAT", bufs=1, side="right")    # attn1
        pV1 = open_pool(name="pV1", bufs=1, side="right")    # all-head V1

        xres_t = pRES.tile([128, MT, QL], F32, tag="xres_t")
        attn1 = pAT.tile([128, MT, QL], F8, tag="attn1")

        # ---- all-head V1 = SV*(x @ wv1 + bv1) in [tokens, d] f8 layout ----
        # moving = weight slices (512 wide), stationary = x token-tiles;
        # the bias lands in PSUM via a ones-row x rv1-row matmul.
        v1all = pV1.tile([128, 16, D], BF16, tag="v1all")
        for st in range(16):
            ts_ = slice(st * 128, st * 128 + 128)
            for db in range(2):
                dsl = slice(db * 512, db * 512 + 512)
                pp = psP.tile([128, 512], F32, tag="pp")
                for m2 in range(MT // 2):
                    nc.tensor.matmul(
                        pp[:, :],
                        xTs[:, 2 * m2:2 * m2 + 2, ts_],
                        wv1s[:, 2 * m2:2 * m2 + 2, dsl],
                        start=(m2 == 0), stop=False,
                        perf_mode=DR,
                    )
                nc.tensor.matmul(pp[:, :], ones_row[:, 0:128], rv1_sb[:, dsl],
                                 start=False, stop=True)
                nc.scalar.activation(v1all[:, st, dsl], pp[:, :], AF.Copy,
                                     scale=DS_XW)

        def make_A(hp):
            ds = slice(hp * 128, hp * 128 + 128)
            wq1s = wpool.tile([128, MT, 128], F8, tag="wq1s")
            wk1s = wpool.tile([128, MT, 128], F8, tag="wk1s")
            for nm, t in (("wq1", wq1s), ("wk1", wk1s)):
                nc.sync.dma_start(out=t, in_=wslices(nm)[:, :, ds])
            KTh = hpool.tile([128, S], BF16, tag="KTh")
            for sb in range(4):
                ss = slice(sb * 512, sb * 512 + 512)
                pp = psP.tile([128, 512], F32, tag="pp")
                for m2 in range(MT // 2):
                    nc.tensor.matmul(
                        pp[:, :],
                        wk1s[:, 2 * m2:2 * m2 + 2, :],
                        xTs[:, 2 * m2:2 * m2 + 2, ss],
                        start=(m2 == 0), stop=(m2 == MT // 2 - 1),
                        perf_mode=DR,
                    )
                nc.scalar.activation(KTh[:, ss], pp[:, :], AF.Identity,
                                     bias=bc["ck1"][:, hp:hp + 1], scale=DS_XW)
            QTh = hpool.tile([128, QL], BF16, tag="QTh")
            for qb in range(2):
                qs = slice(qb * 512, qb * 512 + 512)
                pp = psP.tile([128, 512], F32, tag="pp")
                for m2 in range(MT // 2):
                    nc.tensor.matmul(
                        pp[:, :],
                        wq1s[:, 2 * m2:2 * m2 + 2, :],
                        xqs[:, 2 * m2:2 * m2 + 2, qs],
                        start=(m2 == 0), stop=(m2 == MT // 2 - 1),
                        perf_mode=DR,
                    )
                nc.scalar.activation(QTh[:, qs], pp[:, :], AF.Identity,
                                     bias=bc["cq1"][:, hp:hp + 1], scale=DS_XW)
            VH = hpool.tile([128, 8, 2, 2, 65], BF16, tag="VH")
            nc.vector.memset(VH[:, :, :, :, 64:65], 1.0)
            for st in range(16):
                _eng(nc).tensor_copy(
                    VH[:, st // 2, st % 2, :, 0:64],
                    v1all[:, st, ds].rearrange("p (a b) -> p a b", a=2),
                )
            return KTh, QTh, VH

        preA = make_A(0)
        for hp in range(MT):
            curA = preA
            if hp + 1 < MT:
                preA = make_A(hp + 1)
            attention(curA[0], curA[2], curA[1], attn1, True, hp)
            if hp == 0:
                for mt in range(MT):
                    nc.sync.dma_start(
                        out=xres_t[:, mt, :],
                        in_=xres.ap().rearrange(
                            "(mt p) s -> p mt s", p=128)[:, mt, :],
                    )
        flush_epilogues()

        close_pool("pV1")   # free v1all
        close_pool("pA")  # free xT/xTq/mask

        # ---- enc loads + Q2 projection (independent of attn1) hoisted here
        # so their PE work fills the LN1/V2/AllGather serial window ----
        pENCS = open_pool(name="pENCS", bufs=1)
        pK2 = open_pool(name="pK2", bufs=1)
        pENCQ = open_pool(name="pENCQ", bufs=1)
        encs = pENCS.tile([128, MT, S], F8, tag="encs")
        for mt in range(MT):
            nc.sync.dma_start(
                out=encs[:, mt, :],
                in_=encT.ap().rearrange("(mt p) s -> p mt s", p=128)[:, mt, :],
            )
        encq = pENCQ.tile([128, MT, QL], F8, tag="encq")
        for mt in range(MT):
            nc.sync.dma_start(
                out=encq[:, mt, :],
                in_=encTq.ap().rearrange("(mt p) s -> p mt s", p=128)[:, mt, :],
            )
        Q2T = pK2.tile([128, MT, QL], BF16, tag="Q2T")
        for nt in range(MT):
            nsl = slice(nt * 128, nt * 128 + 128)
            wq2s = wpool.tile([128, MT, 128], F8, tag="wq1s")
            nc.sync.dma_start(out=wq2s, in_=wslices("wq2")[:, :, nsl])
            for qb in range(2):
                qs = slice(qb * 512, qb * 512 + 512)
                pp = psP.tile([128, 512], F32, tag="pp")
                for m2 in range(MT // 2):
                    nc.tensor.matmul(
                        pp[:, :],
                        wq2s[:, 2 * m2:2 * m2 + 2, :],
                        encq[:, 2 * m2:2 * m2 + 2, qs],
                        start=(m2 == 0), stop=(m2 == MT // 2 - 1),
                        perf_mode=DR,
                    )
                nc.scalar.activation(Q2T[:, nt, qs], pp[:, :], AF.Identity,
                                     bias=bc["cq2"][:, nt:nt + 1], scale=DS_XW)
        close_pool("pENCQ")

        # ---- out-proj 1 + residual -> z1, then LN1 -> x1 ----
        # weight tiles prefetched two (qb,nt)-steps ahead so the first
        # matmuls of each step never wait on DMA.
        def dma_wo(name, nt):
            w = wpool.tile([128, MT, 128], F8, tag="wo1s")
            nc.sync.dma_start(
                out=w, in_=wslices(name)[:, :, nt * 128:nt * 128 + 128])
            return w

        pZ = open_pool(name="pZ", bufs=1)
        z1 = pZ.tile([128, MT, QL], F32, tag="z1")
        steps = [(qb, nt) for qb in range(2) for nt in range(MT)]
        wo_pre = [dma_wo("wo1", steps[0][1]), dma_wo("wo1", steps[1][1])]
        for i, (qb, nt) in enumerate(steps):
            qs = slice(qb * 512, qb * 512 + 512)
            wo1s = wo_pre[i % 2]
            pp = psP.tile([128, 512], F32, tag="pp")
            for d2 in range(MT // 2):
                nc.tensor.matmul(pp[:, :], wo1s[:, 2 * d2:2 * d2 + 2, :],
                                 attn1[:, 2 * d2:2 * d2 + 2, qs],
                                 start=(d2 == 0), stop=False, perf_mode=DR)
            nc.tensor.matmul(pp[:, :], ro1_sb[:, nt * 128:nt * 128 + 128],
                             ones_row[:, :], start=False, stop=True)
            nc.vector.scalar_tensor_tensor(
                z1[:, nt, qs], pp[:, :], DS_OW,
                xres_t[:, nt, qs], op0=MUL, op1=ADD,
            )
            if i + 2 < len(steps):
                wo_pre[i % 2] = dma_wo("wo1", steps[i + 2][1])

        close_pool("pAT")   # free attn1 (right stack top)
        close_pool("pRES")  # free xres

        pX1 = open_pool(name="pX1", bufs=1)
        pX1B = open_pool(name="pX1B", bufs=1)
        x1 = pX1.tile([128, MT, QL], F32, tag="x1")
        x1f8 = pX1B.tile([128, MT, QL], F8, tag="x1f8")
        for qb in range(2):
            qs = slice(qb * 512, qb * 512 + 512)
            ln_block(z1[:, :, qs], "g1", "be1", x1, x1f8, qb)

        # ================= V2 projection + split AllGather =================
        pV2 = open_pool(name="pV2", bufs=1)
        wv2s = wpool.tile([128, MT, D], F8, tag="wvfull", bufs=1)
        nc.sync.dma_start(out=wv2s, in_=wslices("wv2"))
        v2sb = pV2.tile([128, MT, D], BF16, tag="v2sb")
        for half, (v2loc_h, v2all_h) in enumerate(
                ((v2locA, v2allA), (v2locB, v2allB))):
            for st in range(4 * half, 4 * half + 4):
                ss = slice(st * 128, st * 128 + 128)
                for db in range(2):
                    dsl = slice(db * 512, db * 512 + 512)
                    pp = psP.tile([128, 512], F32, tag="pp")
                    for m2 in range(MT // 2):
                        nc.tensor.matmul(
                            pp[:, :],
                            x1f8[:, 2 * m2:2 * m2 + 2, ss],
                            wv2s[:, 2 * m2:2 * m2 + 2, dsl],
                            start=(m2 == 0), stop=False,
                            perf_mode=DR,
                        )
                    nc.tensor.matmul(pp[:, :], ones_row[:, 0:128],
                                     rv2_sb[:, dsl], start=False, stop=True)
                    nc.scalar.activation(v2sb[:, st, dsl], pp[:, :],
                                         AF.Copy, scale=DS_AW)
            nc.sync.dma_start(
                out=v2loc_h.ap().rearrange("(st p) d -> p st d", p=128),
                in_=v2sb[:, 4 * half:4 * half + 4, :],
            )
            nc.gpsimd.collective_compute(
                "AllGather",
                mybir.AluOpType.bypass,
                replica_groups=[[2 * p, 2 * p + 1] for p in range(4)],
                ins=[v2loc_h[:, :]],
                outs=[v2all_h[:, :]],
            )
        close_pool("pV2")
        close_pool("pX1B")  # x1f8 only needed for the V2 projection

        # ================= PHASE B: cross-attention =================
        pAT2 = open_pool(name="pAT2", bufs=1)

        attn2 = pAT2.tile([128, MT, QL], F8, tag="attn2")

        def make_B(hp):
            ds = slice(hp * 128, hp * 128 + 128)
            wk2s = wpool.tile([128, MT, 128], F8, tag="wk1s")
            nc.sync.dma_start(out=wk2s, in_=wslices("wk2")[:, :, ds])
            K2h = hpool.tile([128, S], BF16, tag="KTh")
            for sb in range(4):
                ss = slice(sb * 512, sb * 512 + 512)
                pp = psP.tile([128, 512], F32, tag="pp")
                for m2 in range(MT // 2):
                    nc.tensor.matmul(
                        pp[:, :],
                        wk2s[:, 2 * m2:2 * m2 + 2, :],
                        encs[:, 2 * m2:2 * m2 + 2, ss],
                        start=(m2 == 0), stop=(m2 == MT // 2 - 1),
                        perf_mode=DR,
                    )
                nc.scalar.activation(K2h[:, ss], pp[:, :], AF.Identity,
                                     bias=bc["ck2"][:, hp:hp + 1], scale=DS_XW)
            VH2 = hpool.tile([128, 8, 2, 2, 65], BF16, tag="VH")
            nc.vector.memset(VH2[:, :, :, :, 64:65], 1.0)
            for t in range(16):
                if t < 8:
                    vsrc, row0 = v2allA, t * 128
                elif t < 12:
                    vsrc, row0 = v2allB, (t - 4) * 128
                else:
                    vsrc, row0 = v2allB, (t - 12) * 128
                nc.sync.dma_start(
                    out=VH2[:, t // 2, t % 2, :, 0:64],
                    in_=vsrc[row0:row0 + 128,
                             hp * 128:hp * 128 + 128].rearrange(
                        "p (a b) -> p a b", a=2),
                )
            return K2h, VH2

        preB = make_B(0)
        for hp in range(MT):
            curB = preB
            if hp + 1 < MT:
                preB = make_B(hp + 1)
            attention(curB[0], curB[1], Q2T[:, hp, :], attn2, False, hp)

        # ---- out-proj 2 + residual -> z2, then LN2 -> x2 ----
        z2 = pZ.tile([128, MT, QL], F32, tag="z1")
        wo_pre = [dma_wo("wo2", steps[0][1]), dma_wo("wo2", steps[1][1])]
        for i, (qb, nt) in enumerate(steps):
            qs = slice(qb * 512, qb * 512 + 512)
            if i == 2:
                flush_epilogues()
            wo2s = wo_pre[i % 2]
            pp = psP.tile([128, 512], F32, tag="pp")
            for d2 in range(MT // 2):
                nc.tensor.matmul(pp[:, :], wo2s[:, 2 * d2:2 * d2 + 2, :],
                                 attn2[:, 2 * d2:2 * d2 + 2, qs],
                                 start=(d2 == 0), stop=False, perf_mode=DR)
            nc.tensor.matmul(pp[:, :], ro2_sb[:, nt * 128:nt * 128 + 128],
                             ones_row[:, :], start=False, stop=True)
            nc.vector.scalar_tensor_tensor(
                z2[:, nt, qs], pp[:, :], DS_OW,
                x1[:, nt, qs], op0=MUL, op1=ADD,
            )
            if i + 2 < len(steps):
                wo_pre[i % 2] = dma_wo("wo2", steps[i + 2][1])

        close_pool("pAT2")
        close_pool("pX1")

        pX2 = open_pool(name="pX2", bufs=1, side="right")
        pX2B = open_pool(name="pX2B", bufs=1, side="right")
        x2 = pX2.tile([128, MT, QL], F32, tag="x2")
        x2f8 = pX2B.tile([128, MT, QL], F8, tag="x2f8")
        for qb in range(2):
            qs = slice(qb * 512, qb * 512 + 512)
            ln_block(z2[:, :, qs], "g2", "be2", x2, x2f8, qb)
        close_pool("pZ")
        close_pool("pK2")
        close_pool("pENCS")

        # ================= PHASE C: FFN + LN3 -> out =================
        pF = open_pool(name="pF", bufs=1, side="right")
        pF2 = open_pool(name="pF2", bufs=2)
        hT = pF.tile([128, FT, QL], F8, tag="hT")

        def dma_wf1(ft):
            wf1s = pF2.tile([128, MT, 128], F8, tag="wf1s")
            nc.sync.dma_start(
                out=wf1s,
                in_=wd["wf1"].ap().rearrange("(mt p) f -> p mt f", p=128)[
                    :, :, ft * 128:ft * 128 + 128],
            )
            return wf1s

        wf1_pre = dma_wf1(0)
        for ft in range(FT):
            wf1s = wf1_pre
            if ft + 1 < FT:
                wf1_pre = dma_wf1(ft + 1)
            for qb in range(2):
                qs = slice(qb * 512, qb * 512 + 512)
                pp = psP.tile([128, 512], F32, tag="pp")
                for m2 in range(MT // 2):
                    nc.tensor.matmul(
                        pp[:, :],
                        wf1s[:, 2 * m2:2 * m2 + 2, :],
                        x2f8[:, 2 * m2:2 * m2 + 2, qs],
                        start=(m2 == 0), stop=(m2 == MT // 2 - 1),
                        perf_mode=DR,
                    )
                # hT = SA * relu(psum*DS_AW + bf1); cf1 is host-scaled by SA
                nc.scalar.activation(hT[:, ft, qs], pp[:, :], AF.Relu,
                                     bias=bc["cf1"][:, ft:ft + 1],
                                     scale=DS_AW * SA)
        # z3 overwrites x2 in place: x2's last use is this residual add.
        # qb-outer so LN3+store of qb0 overlap FF2 matmuls of qb1.
        def dma_wf2(nt):
            wf2s = pF2.tile([128, FT, 128], F8, tag="wf2s")
            nc.sync.dma_start(
                out=wf2s,
                in_=wd["wf2"].ap().rearrange("(ft p) d -> p ft d", p=128)[
                    :, :, nt * 128:nt * 128 + 128],
            )
            return wf2s

        wf2_pre = dma_wf2(0)
        for qb in range(2):
            qs = slice(qb * 512, qb * 512 + 512)
            for nt in range(MT):
                wf2s = wf2_pre
                if nt + 1 < MT or qb == 0:
                    wf2_pre = dma_wf2((nt + 1) % MT)
                pp = psP.tile([128, 512], F32, tag="pp")
                for f2 in range(FT // 2):
                    nc.tensor.matmul(
                        pp[:, :],
                        wf2s[:, 2 * f2:2 * f2 + 2, :],
                        hT[:, 2 * f2:2 * f2 + 2, qs],
                        start=(f2 == 0), stop=False,
                        perf_mode=DR,
                    )
                nc.tensor.matmul(pp[:, :], rf2_sb[:, nt * 128:nt * 128 + 128],
                                 ones_row[:, :], start=False, stop=True)
                nc.vector.scalar_tensor_tensor(
                    x2[:, nt, qs], pp[:, :], DS_H2,
                    x2[:, nt, qs], op0=MUL, op1=ADD,
                )
            outsb = pF.tile([128, MT, 512], F32, tag="outsb")
            ln_block(x2[:, :, qs], "g3", "be3", outsb, None, 0)
            nc.sync.dma_start(
                out=outT.ap().rearrange("(mt p) q -> p mt q", p=128)[:, :, qs],
                in_=outsb,
            )
        close_pool("pF2")
        close_pool("pF")
        close_pool("pX2B")
        close_pool("pX2")

        for nm in reversed(list(_cms)):
            close_pool(nm)

    return nc


_CACHED = {}


def _get_nc():
    if "nc" not in _CACHED:
        nc = build_nc()
        legalize_waits(nc)
        _CACHED["nc"] = nc
    return _CACHED["nc"]


def _colbias(v, k=8):
    return np.ascontiguousarray(np.asarray(v, np.float32).reshape(k, 128).T)


def _bf(a):
    return np.ascontiguousarray(np.asarray(a)).astype(ml_dtypes.bfloat16)


def _f8(a, scale):
    a = np.asarray(a, np.float32) * scale
    return np.clip(a, -224.0, 224.0).astype(ml_dtypes.float8_e4m3)


def _make_mask(j):
    # ramp[sl][i, u] = (u >= i + c) with c = (896, 896) for j=0 and
    # (384, 1408) for j=1; windows at (896,1408)[sl] - 128*ki reproduce
    # the per-(slot, k-tile) causal masks.
    cs = (896, 896) if j == 0 else (384, 1408)
    i = np.arange(128)[:, None]
    u = np.arange(1920)[None, :]
    m = np.stack([(u >= i + c).astype(np.float32) for c in cs])
    return m.astype(ml_dtypes.bfloat16)


def kernel(**inputs):
    x = np.asarray(inputs["x"], np.float32)
    enc = np.asarray(inputs["encoder_output"], np.float32)
    shared = {}
    for name in ("wq1", "wk1", "wv1", "wq2", "wk2", "wv2"):
        shared[name] = _f8(inputs[name], SW)
    shared["wf1"] = _f8(inputs["wf1"], SW)
    shared["wf2"] = _f8(inputs["wf2"], SW2)
    for name in ("wo1", "wo2"):
        shared[name] = _f8(inputs[name], SWO)
    shared["ro1"] = _bf(np.asarray(inputs["bo1"], np.float32).reshape(1, D)
                        * (SAT * SWO))
    shared["ro2"] = _bf(np.asarray(inputs["bo2"], np.float32).reshape(1, D)
                        * (SAT * SWO))
    shared["rf2"] = _bf(np.asarray(inputs["bf2"], np.float32).reshape(1, D)
                        * (SA * SW2))
    for src, dst in (("bq1", "cq1"), ("bk1", "ck1"), ("bo1", "co1"),
                     ("bq2", "cq2"), ("bk2", "ck2"), ("bo2", "co2"),
                     ("g1", "g1"), ("be1", "be1"), ("g2", "g2"), ("be2", "be2"),
                     ("g3", "g3"), ("be3", "be3")):
        shared[dst] = _colbias(inputs[src], 8)
    shared["cf1"] = _colbias(np.asarray(inputs["bf1"], np.float32) * SA, 32)
    shared["cf2"] = _colbias(inputs["bf2"], 8)
    shared["rv1"] = _bf(np.asarray(inputs["bv1"], np.float32).reshape(1, D)
                        * (SX * SW))
    shared["rv2"] = _bf(np.asarray(inputs["bv2"], np.float32).reshape(1, D)
                        * (SA * SW))
    masks = {0: _make_mask(0), 1: _make_mask(1)}

    in_maps = []
    col_list = []
    for c in range(NCORES):
        b, j = c // 2, c % 2
        q0a, q0b = (0, 1536) if j == 0 else (512, 1024)
        cols = np.r_[q0a:q0a + 512, q0b:q0b + 512]
        col_list.append((b, cols))
        xTb = np.ascontiguousarray(x[b].T)
        encTb = np.ascontiguousarray(enc[b].T)
        m = dict(shared)
        m["xT"] = _f8(xTb, SX)
        m["xTq"] = _f8(xTb[:, cols], SX)
        m["xres"] = np.ascontiguousarray(xTb[:, cols])
        m["encT"] = _f8(encTb, SX)
        m["encTq"] = _f8(encTb[:, cols], SX)
        m["bigmask"] = masks[j]
        in_maps.append(m)

    global _LAST_IN_MAPS
    _LAST_IN_MAPS = in_maps
    nc = _get_nc()
    res = run_bass_kernel_spmd(nc, in_maps, core_ids=list(range(NCORES)))
    out = np.empty((B, S, D), np.float32)
    for c in range(NCORES):
        b, cols = col_list[c]
        out[b, cols, :] = res.results[c]["outT"].T
    return out
